# revision 16
# baseline (speedup 1.0000x reference)
"""DiffPool (nn_DiffPool_4715874091424) Trainium2 Bass kernel.

Math (reference is jax, B=32, C=CR=32, N=L=160, GDEP=2, ALPHA=0.05):
  A  = rownorm(a + I), A' = rownorm(a.T + I)
  mixprop folding:  embed = E0 x + E1 (M1 x) + E2 (M2 x) + 2 be
                    pool  = P0 x + P1 (M1 x) + P2 (M2 x) + 2 bp
  with M1 = A + A', M2 = A^2 + A'^2 (hop matrices), E*/P* folded 32x32
  channel-mix mats (host precompute).
  s = softmax_v(pool);  x_new[c] = s[c]^T @ embed[c];
  a_new[c] = (s[c] @ a) @ s[c].

Device pipeline per batch element b (8 cores, data-parallel over B, 4 b/core):
  1. x node-major (host-pretransposed, contiguous load); y12 = [M1|M2]^T.T @ x
  2. per v-segment (5 x 32 nodes): hcat [96, seg] = [x_chan; y1_chan; y2_chan]
     (y rows via DRAM roundtrip = the layout transpose), channel-mix matmul
     (Wcat [96,64]) + bias -> embed rows / exp(pool - ln64) rows -> DRAM mo
  3. per c-group of 8: reload embed/expP node-major from mo; x_new raw with
     ones-column rhs (softmax denom D rides as col 161); one strided recip
     per l-tile -> Dinv; s^T raw via matmul vs identity rhs, scaled by a
     single stride-0-broadcast tensor_tensor; tT = a^T s^T (const stationary);
     a_new^T raw = expP-stationary @ tT.  Evictions are merged 3-channels-
     per-PSUM-bank plain copies, alternating ACT/DVE.
  Outputs xn / a_new^T / Dinv in fp16/fp32; HOST applies the Dinv row scale
  and the final a_new transpose (softmax shift-invariance makes the -ln64
  shift cancel).  Phase-1 work for b+1 is rationed across phase-2 groups of
  b to keep the PE HAM activity monitor warm; dummy-matmul bursts at batch
  boundaries re-warm the clock gate if a stall window slipped through.
"""

import sys

import numpy as np

if "/opt/trn_rl_repo" not in sys.path:
    sys.path.insert(0, "/opt/trn_rl_repo")

import concourse.bass as bass
import concourse.bacc as bacc
import concourse.mybir as mybir
import concourse.tile as tile
from concourse.bass import AP
from concourse.bass_utils import run_bass_kernel_spmd
from concourse.masks import make_identity

F32 = mybir.dt.float32
F16 = mybir.dt.float16
AF = mybir.ActivationFunctionType
MUL = mybir.AluOpType.mult

B, C, N, L = 32, 32, 160, 160
NCORES = 8
BPC = B // NCORES  # 4 batch elements per core
ALPHA, BETA = 0.05, 0.95
LN_SHIFT = float(np.log(1024.0))  # softmax shift: keeps raw fp16 outputs in range
CL = C * L  # 5120
NSEG = 5
VQ = N // NSEG  # 32 node rows per v-segment
QF = VQ * L  # 5120 free elements per segment
G = 8  # channels per phase-2 group
NGRP = C // G
VT = [(0, 128), (128, 32)]  # partition tiles of the 160 node/cluster dim
TRIPLES = [(0, 3), (3, 3), (6, 2)]  # 3-channel psum-bank packing of G=8


class _Evict:
    """Alternate PSUM evictions between DVE and ACT."""

    def __init__(self, nc):
        self.nc = nc
        self.i = 0

    def copy(self, out, in_):
        if self.i % 2 == 0:
            self.nc.vector.tensor_copy(out, in_)
        else:
            self.nc.scalar.activation(out, in_, AF.Copy)
        self.i += 1


def build_nc():
    nc = bacc.Bacc("TRN2", target_bir_lowering=False, debug=False, num_devices=NCORES)
    xs = nc.declare_dram_parameter("xs", [BPC, C, N, L], F16, isOutput=False)
    xnd = nc.declare_dram_parameter("xnd", [BPC, N, C, L], F16, isOutput=False)
    mt = nc.declare_dram_parameter("mt", [N, 2 * N], F16, isOutput=False)
    wcat = nc.declare_dram_parameter("wcat", [3 * C, 2 * C], F16, isOutput=False)
    b2 = nc.declare_dram_parameter("b2", [2 * C, 1], F32, isOutput=False)
    am = nc.declare_dram_parameter("am", [N, N], F16, isOutput=False)
    xn_out = nc.declare_dram_parameter("xn", [BPC, C, L, L], F16, isOutput=True)
    ant_out = nc.declare_dram_parameter("ant", [BPC, C, L, N], F16, isOutput=True)
    dv_out = nc.declare_dram_parameter("dv", [BPC, L, C], F32, isOutput=True)
    mo = nc.dram_tensor("mo", [BPC, 2 * C, N, L], F16)
    ys = nc.dram_tensor("ys", [BPC, C, 2 * N, L], F16)

    ev = _Evict(nc)

    with tile.TileContext(nc) as tc:
        with (
            tc.tile_pool(name="consts", bufs=1) as pc,
            tc.tile_pool(name="work", bufs=1) as pw,
            tc.tile_pool(name="psum", bufs=1, space="PSUM") as pp,
        ):
            # ---- prefetch b=0 x node-major before anything else ----
            xc = _xc_load(nc, pw, xnd, 0)

            # ---- constants ----
            mt0 = pc.tile([128, 2 * N], F16)
            mt1 = pc.tile([32, 2 * N], F16)
            nc.sync.dma_start(mt0[:], mt[0:128, :])
            nc.sync.dma_start(mt1[:], mt[128:160, :])
            wc = pc.tile([3 * C, 2 * C], F16)
            nc.sync.dma_start(wc[:], wcat[:])
            b2c = pc.tile([2 * C, 1], F32)
            nc.sync.dma_start(b2c[:], b2[:])
            am0 = pc.tile([128, N], F16)
            am1 = pc.tile([32, N], F16)
            nc.sync.dma_start(am0[:], am[0:128, :])
            nc.sync.dma_start(am1[:], am[128:160, :])
            # identity-rhs blocks with a trailing ones column: the s^T
            # matmul-transpose then also emits the softmax denominator D as
            # output column 160 (accumulated over both K tiles).
            id0 = pc.tile([128, N + 2], F16, name="id0")
            id1 = pc.tile([32, N + 2], F16, name="id1")
            nc.gpsimd.memset(id0[:], 0.0)
            nc.gpsimd.memset(id1[:], 0.0)
            make_identity(nc, id0[:, 0:128], nomemset=True)
            make_identity(nc, id1[:, 128:160], nomemset=True)
            nc.vector.memset(id0[:, N : N + 1], 1.0)
            nc.vector.memset(id1[:, N : N + 1], 1.0)
            idk = [id0, id1]

            MTILES = [(0, 128), (128, 128), (256, 64)]
            mtt = []
            for kt, (ksz, mtsrc) in enumerate(((128, mt0), (32, mt1))):
                row = []
                for m0, msz in MTILES:
                    t = pc.tile([ksz, msz], F16, name=f"mtt{kt}_{m0}")
                    nc.vector.tensor_copy(t[:], mtsrc[:, m0 : m0 + msz])
                    row.append(t)
                mtt.append(row)
            amt = []
            for kt, (ksz, asrc) in enumerate(((128, am0), (32, am1))):
                row = []
                for m0, msz in VT:
                    t = pc.tile([ksz, msz], F16, name=f"amt{kt}_{m0}")
                    nc.vector.tensor_copy(t[:], asrc[:, m0 : m0 + msz])
                    row.append(t)
                amt.append(row)

            warm = pc.tile([128, 512], F16, name="warm")
            nc.vector.memset(warm[:], 0.125)
            _warm_burst(nc, pp, warm, 28)

            st = {"mtt": mtt, "wc": wc, "b2c": b2c, "amt": amt, "idk": idk,
                  "warm": warm, "xc": {0: xc}}

            # ---- prime: full phase1(b=0) ----
            y = _y12(nc, pw, pp, st, 0)
            _ys_dma(nc, ys, y, 0)
            for q in range(NSEG):
                _mix_seg(nc, pw, pp, st, xs, ys, mo, 0, q)
            ld = _ph2_loads(nc, pw, mo, 0, 0)

            # ---- steady state: phase2(b) groups with phase1(b+1) rationed ----
            for b in range(BPC):
                dvall = [
                    pw.tile([sz, C], F32, tag=f"dva{i}", name=f"dva{i}", bufs=2)
                    for i, (_, sz) in enumerate(VT)
                ]
                slg = _ph2_t1(nc, pw, pp, st, ld, dvall, b, 0, ev)
                for g in range(NGRP):
                    if b + 1 < BPC:
                        if g == 0:
                            st["xc"][b + 1] = _xc_load(nc, pw, xnd, b + 1)
                            yb = _y12_part(nc, pw, pp, st, b + 1, 0)
                        elif g == 1:
                            _y12_part(nc, pw, pp, st, b + 1, 1, yb)
                        elif g == 2:
                            _y12_part(nc, pw, pp, st, b + 1, 2, yb)
                            _ys_dma(nc, ys, yb, b + 1)
                        elif g == 3:
                            for q in range(NSEG):
                                _mix_seg(nc, pw, pp, st, xs, ys, mo, b + 1, q)
                    if g == 0:
                        _warm_burst(nc, pp, warm, 8)  # re-warm insurance
                    nxt = (b, g + 1) if g + 1 < NGRP else (b + 1, 0)
                    ld_next = (
                        _ph2_loads(nc, pw, mo, nxt[0], nxt[1])
                        if nxt[0] < BPC
                        else None
                    )
                    # next group's s^T pass fills this group's recip/scale wait
                    slg_next = (
                        _ph2_t1(nc, pw, pp, st, ld_next, dvall, b, g + 1, ev)
                        if g + 1 < NGRP
                        else None
                    )
                    _ph2_rest(nc, pw, pp, st, ld, slg, xn_out, ant_out, b, g, ev)
                    ld = ld_next
                    slg = slg_next
                for i, (v0, sz) in enumerate(VT):
                    nc.scalar.dma_start(dv_out[b][v0 : v0 + sz, :], dvall[i][:])

    return nc


def _psA(pp, shape, dt=F32):
    return pp.tile(shape, dt, tag="psA", name="psA", bufs=4)


def _psB(pp, shape, dt=F32):
    return pp.tile(shape, dt, tag="psB", name="psB", bufs=4)


def _warm_burst(nc, pp, warm, n):
    for _ in range(n):
        wps = _psA(pp, [128, 512])
        nc.tensor.matmul(wps[:], warm[:, 0:128], warm[:], start=True, stop=True)


def _xc_load(nc, pw, xnd, b):
    xc0 = pw.tile([128, CL], F16, tag="xc0", name="xc0", bufs=2)
    xc1 = pw.tile([32, CL], F16, tag="xc1", name="xc1", bufs=2)
    nc.sync.dma_start(xc0[:], xnd[b, 0:128].rearrange("w c l -> w (c l)"))
    nc.sync.dma_start(xc1[:], xnd[b, 128:160].rearrange("w c l -> w (c l)"))
    return [xc0, xc1]


def _y12_alloc(pw):
    return [
        pw.tile([128, CL], F16, tag="y0", name="y0", bufs=1),
        pw.tile([128, CL], F16, tag="y1", name="y1", bufs=1),
        pw.tile([64, CL], F16, tag="y2", name="y2", bufs=1),
    ]


def _y12_part(nc, pw, pp, st, b, mi, Y=None):
    """One M-tile of the y12 node matmul."""
    MTILES = [(0, 128), (128, 128), (256, 64)]
    if Y is None:
        Y = _y12_alloc(pw)
    xcs = st["xc"][b]
    mtt = st["mtt"]
    m0, msz = MTILES[mi]
    for sg in range(0, 10, 4):  # chunk groups of <=4 (512 cols each)
        subs = range(sg, min(sg + 4, 10))
        pss = {sub: _psA(pp, [128, 512]) for sub in subs}
        for kt in range(2):
            for sub in subs:
                nc.tensor.matmul(
                    pss[sub][:msz, :],
                    mtt[kt][mi][:],
                    xcs[kt][:, sub * 512 : (sub + 1) * 512],
                    start=(kt == 0),
                    stop=(kt == 1),
                )
        for j, sub in enumerate(subs):
            if sub % 2 == 0:
                nc.vector.tensor_copy(
                    Y[mi][:msz, sub * 512 : (sub + 1) * 512], pss[sub][:msz, :]
                )
            else:
                nc.scalar.activation(
                    Y[mi][:msz, sub * 512 : (sub + 1) * 512],
                    pss[sub][:msz, :],
                    AF.Copy,
                )
    return Y


def _y12(nc, pw, pp, st, b):
    Y = _y12_alloc(pw)
    for mi in range(3):
        _y12_part(nc, pw, pp, st, b, mi, Y)
    return Y


def _ys_dma(nc, ys, Y, b):
    MTILES = [(0, 128), (128, 128), (256, 64)]
    for mi, (m0, msz) in enumerate(MTILES):
        nc.gpsimd.dma_start(
            ys[b][:, m0 : m0 + msz, :].rearrange("c v l -> v c l"),
            Y[mi][:].rearrange("v (c l) -> v c l", c=C),
        )


def _mix_seg(nc, pw, pp, st, xs, ys, mo, b, q):
    """hcat assembly + channel mix for one v-segment."""
    wc, b2c = st["wc"], st["b2c"]
    v0 = q * VQ
    hq = pw.tile([3 * C, QF], F16, tag="hcat", name="hcat", bufs=2)
    nc.sync.dma_start(
        hq[0:C, :].rearrange("c (v l) -> c v l", v=VQ),
        xs[b][:, v0 : v0 + VQ, :],
    )
    for blk, base in ((1, 0), (2, N)):
        nc.sync.dma_start(
            hq[blk * C : (blk + 1) * C, :].rearrange("c (v l) -> c v l", v=VQ),
            ys[b][:, base + v0 : base + v0 + VQ, :],
        )
    moq = pw.tile([2 * C, QF], F16, tag="moq", name="moq", bufs=2)
    for off in range(0, QF, 512):
        ps = _psB(pp, [64, 512])
        nc.tensor.matmul(ps[:], wc[:], hq[:, off : off + 512], start=True, stop=True)
        nc.vector.tensor_scalar_add(moq[0:C, off : off + 512], ps[0:C, :], b2c[0:C, :])
        nc.scalar.activation(
            moq[C : 2 * C, off : off + 512],
            ps[C : 2 * C, :],
            AF.Exp,
            bias=b2c[C : 2 * C, :],
        )
    nc.gpsimd.dma_start(
        mo[b][:, v0 : v0 + VQ, :],
        moq[:].rearrange("o (v l) -> o v l", v=VQ),
    )


def _ph2_loads(nc, pw, mo, b, g):
    """Load embed and expP node-major tiles for one c-group."""
    c0 = g * G
    egs, xps = [], []
    for i, (v0, sz) in enumerate(VT):
        eg = pw.tile([sz, G * L], F16, tag=f"eg{i}", name=f"eg{i}", bufs=2)
        xp = pw.tile([sz, G * L], F16, tag=f"xp{i}", name=f"xp{i}", bufs=2)
        nc.sync.dma_start(
            eg[:].rearrange("v (c l) -> v c l", c=G),
            mo[b][c0 : c0 + G, v0 : v0 + sz, :].rearrange("c v l -> v c l"),
        )
        nc.sync.dma_start(
            xp[:].rearrange("v (c l) -> v c l", c=G),
            mo[b][C + c0 : C + c0 + G, v0 : v0 + sz, :].rearrange("c v l -> v c l"),
        )
        egs.append(eg)
        xps.append(xp)
    return egs, xps


def _bcast(ap2d, nfree):
    """Append a stride-0 free dim of size nfree to a [p, c] AP."""
    return AP(ap2d.tensor, ap2d.offset, list(ap2d.ap) + [[0, nfree]])


def _ph2_t1(nc, pw, pp, st, ld, dvall, b, g, ev):
    """s^T raw (+ D in col 160) via matmul vs identity|ones rhs, Dinv, scale.

    Emitted one group ahead of _ph2_rest so the recip/scale latency is
    covered by the next group's PE work (the PE queue is strict FIFO)."""
    idk = st["idk"]
    egs, xps = ld
    c0 = g * G
    SW = N + 2  # even channel stride for the s^T-raw + D layout (psum 8B lines)
    slgr = [
        pw.tile([sz, G * SW], F16, tag=f"sr{i}", name=f"sr{i}", bufs=2)
        for i, (_, sz) in enumerate(VT)
    ]
    slg = [
        pw.tile([sz, G * N], F16, tag=f"sl{i}", name=f"sl{i}", bufs=2)
        for i, (_, sz) in enumerate(VT)
    ]
    for lt, (l0, lsz) in enumerate(VT):  # output l tiles
        for t0, nch in TRIPLES:
            ps = _psB(pp, [128, 512])
            for j in range(nch):
                ci = t0 + j
                for kt, (k0, ksz) in enumerate(VT):  # v tiles (contraction)
                    nc.tensor.matmul(
                        ps[:lsz, j * SW : (j + 1) * SW],
                        xps[kt][:, ci * L + l0 : ci * L + l0 + lsz],
                        idk[kt][:],
                        start=(kt == 0),
                        stop=(kt == 1),
                    )
            ev.copy(
                slgr[lt][:lsz, t0 * SW : (t0 + nch) * SW], ps[:lsz, 0 : nch * SW]
            )
        # one strided reciprocal per l-tile: Dinv for all 8 channels
        nc.vector.reciprocal(
            dvall[lt][:lsz, c0 : c0 + G],
            slgr[lt][:lsz].rearrange("p (c q) -> p c q", c=G)[:, :, N],
        )
        nc.vector.tensor_tensor(
            slg[lt][:lsz].rearrange("p (c v) -> p c v", c=G),
            slgr[lt][:lsz].rearrange("p (c q) -> p c q", c=G)[:, :, 0:N],
            _bcast(dvall[lt][:lsz, c0 : c0 + G], N),
            MUL,
        )
    return slg


def _ph2_rest(nc, pw, pp, st, ld, slg, xn_out, ant_out, b, g, ev):
    amt = st["amt"]
    egs, xps = ld
    c0 = g * G

    xraw = [
        pw.tile([sz, G * L], F16, tag=f"xr{i}", name=f"xr{i}", bufs=2)
        for i, (_, sz) in enumerate(VT)
    ]
    ttg = [
        pw.tile([sz, G * N], F16, tag=f"tt{i}", name=f"tt{i}", bufs=2)
        for i, (_, sz) in enumerate(VT)
    ]
    angr = [
        pw.tile([sz, G * N], F16, tag=f"ag{i}", name=f"ag{i}", bufs=2)
        for i, (_, sz) in enumerate(VT)
    ]

    # ---- tT = a^T s^T ----
    for mi, (m0, msz) in enumerate(VT):  # j tiles
        for cf0, csz in ((0, 512), (512, 512), (1024, 256)):
            ps = _psB(pp, [128, 512])
            for kt in range(2):
                nc.tensor.matmul(
                    ps[:msz, :csz],
                    amt[kt][mi][:],
                    slg[kt][:, cf0 : cf0 + csz],
                    start=(kt == 0),
                    stop=(kt == 1),
                )
            ev.copy(ttg[mi][:msz, cf0 : cf0 + csz], ps[:msz, :csz])

    # ---- x_new raw + a_new^T raw, interleaved: adjacent matmuls share the
    # same expP stationary slice, doubling moving data per weight load ----
    for mi, (m0, msz) in enumerate(VT):  # l tiles (output partition)
        for t0, nch in TRIPLES:
            psx = _psA(pp, [128, 512])
            psa = _psA(pp, [128, 512])
            for j in range(nch):
                ci = t0 + j
                for kt, (k0, ksz) in enumerate(VT):  # v/j tiles (contraction)
                    stat = xps[kt][:, ci * L + m0 : ci * L + m0 + msz]
                    nc.tensor.matmul(
                        psx[:msz, j * L : (j + 1) * L],
                        stat,
                        egs[kt][:, ci * L : (ci + 1) * L],
                        start=(kt == 0),
                        stop=(kt == 1),
                    )
                    nc.tensor.matmul(
                        psa[:msz, j * N : (j + 1) * N],
                        stat,
                        ttg[kt][:, ci * N : (ci + 1) * N],
                        start=(kt == 0),
                        stop=(kt == 1),
                    )
            ev.copy(xraw[mi][:msz, t0 * L : (t0 + nch) * L], psx[:msz, 0 : nch * L])
            ev.copy(angr[mi][:msz, t0 * N : (t0 + nch) * N], psa[:msz, 0 : nch * N])

    # ---- outputs (raw fp16; host applies Dinv scale / transpose) ----
    for i, (v0, sz) in enumerate(VT):
        nc.scalar.dma_start(
            xn_out[b][c0 : c0 + G, v0 : v0 + sz, :].rearrange("c p q -> p c q"),
            xraw[i][:sz].rearrange("p (c q) -> p c q", c=G),
        )
        nc.scalar.dma_start(
            ant_out[b][c0 : c0 + G, v0 : v0 + sz, :].rearrange("c p q -> p c q"),
            angr[i][:sz].rearrange("p (c q) -> p c q", c=G),
        )


def _host_prep(x, a, We, be, Wp, bp):
    a = np.asarray(a, np.float64)
    I = np.eye(N, dtype=np.float64)
    A1 = (a + I) / (a + I).sum(1, keepdims=True)
    A2 = (a.T + I) / (a.T + I).sum(1, keepdims=True)
    M1 = A1 + A2
    M2 = A1 @ A1 + A2 @ A2
    MT = np.concatenate([M1.T, M2.T], axis=1).astype(np.float16)  # [N, 2N]

    def fold(W):
        W = np.asarray(W, np.float64)
        W0, W1, W2 = W[:, :C], W[:, C : 2 * C], W[:, 2 * C :]
        F0 = 2.0 * (W0 + ALPHA * W1 + ALPHA * W2)
        F1 = BETA * W1 + ALPHA * BETA * W2
        F2 = BETA * BETA * W2
        return F0, F1, F2

    E0, E1, E2 = fold(We)
    P0, P1, P2 = fold(Wp)
    Wcat = np.block([[E0.T, P0.T], [E1.T, P1.T], [E2.T, P2.T]]).astype(np.float16)
    b2 = np.concatenate(
        [2.0 * np.asarray(be), 2.0 * np.asarray(bp) - LN_SHIFT]
    ).astype(np.float32)[:, None]
    return MT, Wcat, b2, np.asarray(a, np.float16)


def _postprocess(xn_raw, ant_raw, dv):
    # dv: [*, L, C] Dinv values; raw outputs are scaled by Dinv along their
    # l (row) dim, then a_new^T is transposed back.
    dinv = dv.transpose(0, 2, 1)[:, :, :, None]  # [*, C, L, 1]
    xn = xn_raw.astype(np.float32) * dinv
    an = (ant_raw.astype(np.float32) * dinv).swapaxes(-1, -2)
    return np.ascontiguousarray(xn), np.ascontiguousarray(an)


def _install_ntff_shim():
    """Provide antenv.axon_hooks (missing in this image) so
    run_bass_kernel_spmd(trace=True) can drive NTFF profiling via the
    axon PJRT .so. No-op if anything is unavailable."""
    import contextlib
    import ctypes
    import types

    try:
        import antenv  # noqa: F401

        try:
            from antenv.axon_hooks import get_axon_ntff_profile_hook  # noqa: F401

            return
        except ImportError:
            pass
        lib = ctypes.CDLL("/opt/axon/libaxon_pjrt.so")
        if not hasattr(lib, "axon_start_nrt_profile"):
            return
        lib.axon_start_nrt_profile.argtypes = [
            ctypes.POINTER(ctypes.c_int64),
            ctypes.c_size_t,
        ]
        lib.axon_start_nrt_profile.restype = ctypes.c_int64
        lib.axon_stop_nrt_profile.argtypes = [ctypes.c_char_p]
        lib.axon_stop_nrt_profile.restype = ctypes.c_int64

        @contextlib.contextmanager
        def _hook(output_dir, device_ids):
            import jax

            jax.devices()
            if device_ids:
                ids = (ctypes.c_int64 * len(device_ids))(*device_ids)
                rc = lib.axon_start_nrt_profile(ids, len(device_ids))
            else:
                rc = lib.axon_start_nrt_profile(None, 0)
            if rc != 0:
                raise RuntimeError(f"axon_start_nrt_profile rc={rc}")
            try:
                yield
            finally:
                n = lib.axon_stop_nrt_profile(str(output_dir).encode())
                print(f"ntff profile: {n} file(s) -> {output_dir}", file=sys.stderr)

        holder = {"h": _hook}
        mod = types.ModuleType("antenv.axon_hooks")
        mod.get_axon_ntff_profile_hook = lambda: holder["h"]
        mod.set_axon_ntff_profile_hook = lambda h: holder.__setitem__("h", h)
        sys.modules["antenv.axon_hooks"] = mod
        antenv.axon_hooks = mod
    except Exception as e:  # pragma: no cover
        print(f"ntff shim unavailable: {e}", file=sys.stderr)


_NC_CACHE = {}


def _get_nc():
    if "nc" not in _NC_CACHE:
        nc = build_nc()
        nc.compile()
        _NC_CACHE["nc"] = nc
    return _NC_CACHE["nc"]


def run_spmd(x, a, We, be, Wp, bp, trace=False):
    if trace:
        _install_ntff_shim()
    x16 = np.ascontiguousarray(np.asarray(x, np.float16))
    xnd = np.ascontiguousarray(x16.transpose(0, 2, 1, 3))  # [B, N, C, L]
    MT, Wcat, b2, a16 = _host_prep(x, a, We, be, Wp, bp)
    nc = _get_nc()
    in_maps = [
        {
            "xs": x16[i * BPC : (i + 1) * BPC],
            "xnd": xnd[i * BPC : (i + 1) * BPC],
            "mt": MT,
            "wcat": Wcat,
            "b2": b2,
            "am": a16,
        }
        for i in range(NCORES)
    ]
    res = run_bass_kernel_spmd(nc, in_maps, list(range(NCORES)), trace=trace)
    xn_raw = np.concatenate([res.results[i]["xn"] for i in range(NCORES)], axis=0)
    ant_raw = np.concatenate([res.results[i]["ant"] for i in range(NCORES)], axis=0)
    dv = np.concatenate([res.results[i]["dv"] for i in range(NCORES)], axis=0)
    xn, an = _postprocess(xn_raw, ant_raw, dv)
    return (xn, an), res


def kernel(x, a, We, be, Wp, bp):
    (xn, an), _ = run_spmd(x, a, We, be, Wp, bp, trace=False)
    return (xn, an)


# revision 18
# speedup vs baseline: 1.0056x; 1.0056x over previous
"""DiffPool (nn_DiffPool_4715874091424) Trainium2 Bass kernel.

Math (reference is jax, B=32, C=CR=32, N=L=160, GDEP=2, ALPHA=0.05):
  A  = rownorm(a + I), A' = rownorm(a.T + I)
  mixprop folding:  embed = E0 x + E1 (M1 x) + E2 (M2 x) + 2 be
                    pool  = P0 x + P1 (M1 x) + P2 (M2 x) + 2 bp
  with M1 = A + A', M2 = A^2 + A'^2 (hop matrices), E*/P* folded 32x32
  channel-mix mats (host precompute).
  s = softmax_v(pool);  x_new[c] = s[c]^T @ embed[c];
  a_new[c] = (s[c] @ a) @ s[c].

Device pipeline per batch element b (8 cores, data-parallel over B, 4 b/core):
  1. x node-major (host-pretransposed, contiguous load); y12 = [M1|M2]^T.T @ x
  2. per v-segment (5 x 32 nodes): hcat [96, seg] = [x_chan; y1_chan; y2_chan]
     (y rows via DRAM roundtrip = the layout transpose), channel-mix matmul
     (Wcat [96,64]) + bias -> embed rows / exp(pool - ln64) rows -> DRAM mo
  3. per c-group of 8: reload embed/expP node-major from mo; x_new raw with
     ones-column rhs (softmax denom D rides as col 161); one strided recip
     per l-tile -> Dinv; s^T raw via matmul vs identity rhs, scaled by a
     single stride-0-broadcast tensor_tensor; tT = a^T s^T (const stationary);
     a_new^T raw = expP-stationary @ tT.  Evictions are merged 3-channels-
     per-PSUM-bank plain copies, alternating ACT/DVE.
  Outputs xn / a_new^T / Dinv in fp16/fp32; HOST applies the Dinv row scale
  and the final a_new transpose (softmax shift-invariance makes the -ln64
  shift cancel).  Phase-1 work for b+1 is rationed across phase-2 groups of
  b to keep the PE HAM activity monitor warm; dummy-matmul bursts at batch
  boundaries re-warm the clock gate if a stall window slipped through.
"""

import sys

import numpy as np

if "/opt/trn_rl_repo" not in sys.path:
    sys.path.insert(0, "/opt/trn_rl_repo")

import concourse.bass as bass
import concourse.bacc as bacc
import concourse.mybir as mybir
import concourse.tile as tile
from concourse.bass import AP
from concourse.bass_utils import run_bass_kernel_spmd
from concourse.masks import make_identity

F32 = mybir.dt.float32
F16 = mybir.dt.float16
AF = mybir.ActivationFunctionType
MUL = mybir.AluOpType.mult

B, C, N, L = 32, 32, 160, 160
NCORES = 8
BPC = B // NCORES  # 4 batch elements per core
ALPHA, BETA = 0.05, 0.95
LN_SHIFT = float(np.log(1024.0))  # softmax shift: keeps raw fp16 outputs in range
CL = C * L  # 5120
NSEG = 5
VQ = N // NSEG  # 32 node rows per v-segment
QF = VQ * L  # 5120 free elements per segment
G = 8  # channels per phase-2 group
NGRP = C // G
VT = [(0, 128), (128, 32)]  # partition tiles of the 160 node/cluster dim
TRIPLES = [(0, 3), (3, 3), (6, 2)]  # 3-channel psum-bank packing of G=8


class _Evict:
    """Alternate PSUM evictions between DVE and ACT."""

    def __init__(self, nc):
        self.nc = nc
        self.i = 0

    def copy(self, out, in_):
        if self.i % 2 == 0:
            self.nc.vector.tensor_copy(out, in_)
        else:
            self.nc.scalar.activation(out, in_, AF.Copy)
        self.i += 1


def build_nc():
    nc = bacc.Bacc("TRN2", target_bir_lowering=False, debug=False, num_devices=NCORES)
    xs = nc.declare_dram_parameter("xs", [BPC, C, N, L], F16, isOutput=False)
    xnd = nc.declare_dram_parameter("xnd", [BPC, N, C, L], F16, isOutput=False)
    mt = nc.declare_dram_parameter("mt", [N, 2 * N], F16, isOutput=False)
    wcat = nc.declare_dram_parameter("wcat", [3 * C, 2 * C], F16, isOutput=False)
    b2 = nc.declare_dram_parameter("b2", [2 * C, 1], F32, isOutput=False)
    am = nc.declare_dram_parameter("am", [N, N], F16, isOutput=False)
    xn_out = nc.declare_dram_parameter("xn", [BPC, C, L, L], F16, isOutput=True)
    ant_out = nc.declare_dram_parameter("ant", [BPC, C, L, N], F16, isOutput=True)
    dv_out = nc.declare_dram_parameter("dv", [BPC, L, C], F32, isOutput=True)
    mo = nc.dram_tensor("mo", [BPC, 2 * C, N, L], F16)
    ys = nc.dram_tensor("ys", [BPC, C, 2 * N, L], F16)

    ev = _Evict(nc)

    with tile.TileContext(nc) as tc:
        with (
            tc.tile_pool(name="consts", bufs=1) as pc,
            tc.tile_pool(name="work", bufs=1) as pw,
            tc.tile_pool(name="psum", bufs=1, space="PSUM") as pp,
        ):
            # ---- prefetch b=0 x node-major before anything else ----
            xc = _xc_load(nc, pw, xnd, 0)

            # ---- constants ----
            mt0 = pc.tile([128, 2 * N], F16)
            mt1 = pc.tile([32, 2 * N], F16)
            nc.sync.dma_start(mt0[:], mt[0:128, :])
            nc.sync.dma_start(mt1[:], mt[128:160, :])
            wc = pc.tile([3 * C, 2 * C], F16)
            nc.sync.dma_start(wc[:], wcat[:])
            b2c = pc.tile([2 * C, 1], F32)
            nc.sync.dma_start(b2c[:], b2[:])
            am0 = pc.tile([128, N], F16)
            am1 = pc.tile([32, N], F16)
            nc.sync.dma_start(am0[:], am[0:128, :])
            nc.sync.dma_start(am1[:], am[128:160, :])
            # identity-rhs blocks with a trailing ones column: the s^T
            # matmul-transpose then also emits the softmax denominator D as
            # output column 160 (accumulated over both K tiles).
            id0 = pc.tile([128, N + 2], F16, name="id0")
            id1 = pc.tile([32, N + 2], F16, name="id1")
            nc.gpsimd.memset(id0[:], 0.0)
            nc.gpsimd.memset(id1[:], 0.0)
            make_identity(nc, id0[:, 0:128], nomemset=True)
            make_identity(nc, id1[:, 128:160], nomemset=True)
            nc.vector.memset(id0[:, N : N + 1], 1.0)
            nc.vector.memset(id1[:, N : N + 1], 1.0)
            idk = [id0, id1]

            MTILES = [(0, 128), (128, 128), (256, 64)]
            mtt = []
            for kt, (ksz, mtsrc) in enumerate(((128, mt0), (32, mt1))):
                row = []
                for m0, msz in MTILES:
                    t = pc.tile([ksz, msz], F16, name=f"mtt{kt}_{m0}")
                    nc.vector.tensor_copy(t[:], mtsrc[:, m0 : m0 + msz])
                    row.append(t)
                mtt.append(row)
            amt = []
            for kt, (ksz, asrc) in enumerate(((128, am0), (32, am1))):
                row = []
                for m0, msz in VT:
                    t = pc.tile([ksz, msz], F16, name=f"amt{kt}_{m0}")
                    nc.vector.tensor_copy(t[:], asrc[:, m0 : m0 + msz])
                    row.append(t)
                amt.append(row)

            warm = pc.tile([128, 512], F16, name="warm")
            nc.vector.memset(warm[:], 0.125)
            _warm_burst(nc, pp, warm, 28)

            st = {"mtt": mtt, "wc": wc, "b2c": b2c, "amt": amt, "idk": idk,
                  "warm": warm, "xc": {0: xc}}

            # ---- prime: full phase1(b=0) ----
            y = _y12(nc, pw, pp, st, 0)
            _ys_dma(nc, ys, y, 0)
            for q in range(NSEG):
                _mix_seg(nc, pw, pp, st, xs, ys, mo, 0, q)
            ld = _ph2_loads(nc, pw, mo, 0, 0)

            # ---- steady state: phase2(b) groups with phase1(b+1) rationed ----
            for b in range(BPC):
                dvall = [
                    pw.tile([sz, C], F32, tag=f"dva{i}", name=f"dva{i}", bufs=2)
                    for i, (_, sz) in enumerate(VT)
                ]
                for g in range(NGRP):
                    if b + 1 < BPC:
                        if g == 0:
                            st["xc"][b + 1] = _xc_load(nc, pw, xnd, b + 1)
                            yb = _y12_part(nc, pw, pp, st, b + 1, 0)
                        elif g == 1:
                            _y12_part(nc, pw, pp, st, b + 1, 1, yb)
                        elif g == 2:
                            _y12_part(nc, pw, pp, st, b + 1, 2, yb)
                            _ys_dma(nc, ys, yb, b + 1)
                        elif g == 3:
                            for q in range(NSEG):
                                _mix_seg(nc, pw, pp, st, xs, ys, mo, b + 1, q)
                    if g == 0:
                        _warm_burst(nc, pp, warm, 8)  # re-warm insurance
                    nxt = (b, g + 1) if g + 1 < NGRP else (b + 1, 0)
                    ld_next = (
                        _ph2_loads(nc, pw, mo, nxt[0], nxt[1])
                        if nxt[0] < BPC
                        else None
                    )
                    _ph2_group(nc, pw, pp, st, ld, dvall, xn_out, ant_out, b, g, ev)
                    ld = ld_next
                for i, (v0, sz) in enumerate(VT):
                    nc.scalar.dma_start(dv_out[b][v0 : v0 + sz, :], dvall[i][:])

    return nc


def _psA(pp, shape, dt=F32):
    return pp.tile(shape, dt, tag="psA", name="psA", bufs=4)


def _psB(pp, shape, dt=F32):
    return pp.tile(shape, dt, tag="psB", name="psB", bufs=4)


def _warm_burst(nc, pp, warm, n):
    for _ in range(n):
        wps = _psA(pp, [128, 512])
        nc.tensor.matmul(wps[:], warm[:, 0:128], warm[:], start=True, stop=True)


def _xc_load(nc, pw, xnd, b):
    xc0 = pw.tile([128, CL], F16, tag="xc0", name="xc0", bufs=2)
    xc1 = pw.tile([32, CL], F16, tag="xc1", name="xc1", bufs=2)
    nc.sync.dma_start(xc0[:], xnd[b, 0:128].rearrange("w c l -> w (c l)"))
    nc.sync.dma_start(xc1[:], xnd[b, 128:160].rearrange("w c l -> w (c l)"))
    return [xc0, xc1]


def _y12_alloc(pw):
    return [
        pw.tile([128, CL], F16, tag="y0", name="y0", bufs=1),
        pw.tile([128, CL], F16, tag="y1", name="y1", bufs=1),
        pw.tile([64, CL], F16, tag="y2", name="y2", bufs=1),
    ]


def _y12_part(nc, pw, pp, st, b, mi, Y=None):
    """One M-tile of the y12 node matmul."""
    MTILES = [(0, 128), (128, 128), (256, 64)]
    if Y is None:
        Y = _y12_alloc(pw)
    xcs = st["xc"][b]
    mtt = st["mtt"]
    m0, msz = MTILES[mi]
    for sg in range(0, 10, 4):  # chunk groups of <=4 (512 cols each)
        subs = range(sg, min(sg + 4, 10))
        pss = {sub: _psA(pp, [128, 512]) for sub in subs}
        for kt in range(2):
            for sub in subs:
                nc.tensor.matmul(
                    pss[sub][:msz, :],
                    mtt[kt][mi][:],
                    xcs[kt][:, sub * 512 : (sub + 1) * 512],
                    start=(kt == 0),
                    stop=(kt == 1),
                )
        for j, sub in enumerate(subs):
            if sub % 2 == 0:
                nc.vector.tensor_copy(
                    Y[mi][:msz, sub * 512 : (sub + 1) * 512], pss[sub][:msz, :]
                )
            else:
                nc.scalar.activation(
                    Y[mi][:msz, sub * 512 : (sub + 1) * 512],
                    pss[sub][:msz, :],
                    AF.Copy,
                )
    return Y


def _y12(nc, pw, pp, st, b):
    Y = _y12_alloc(pw)
    for mi in range(3):
        _y12_part(nc, pw, pp, st, b, mi, Y)
    return Y


def _ys_dma(nc, ys, Y, b):
    MTILES = [(0, 128), (128, 128), (256, 64)]
    for mi, (m0, msz) in enumerate(MTILES):
        nc.gpsimd.dma_start(
            ys[b][:, m0 : m0 + msz, :].rearrange("c v l -> v c l"),
            Y[mi][:].rearrange("v (c l) -> v c l", c=C),
        )


def _mix_seg(nc, pw, pp, st, xs, ys, mo, b, q):
    """hcat assembly + channel mix for one v-segment."""
    wc, b2c = st["wc"], st["b2c"]
    v0 = q * VQ
    hq = pw.tile([3 * C, QF], F16, tag="hcat", name="hcat", bufs=2)
    nc.sync.dma_start(
        hq[0:C, :].rearrange("c (v l) -> c v l", v=VQ),
        xs[b][:, v0 : v0 + VQ, :],
    )
    for blk, base in ((1, 0), (2, N)):
        nc.sync.dma_start(
            hq[blk * C : (blk + 1) * C, :].rearrange("c (v l) -> c v l", v=VQ),
            ys[b][:, base + v0 : base + v0 + VQ, :],
        )
    moq = pw.tile([2 * C, QF], F16, tag="moq", name="moq", bufs=2)
    for off in range(0, QF, 1024):
        # two 512-col chunks col-packed into array col-groups 0 / 64: the
        # M=64 matmuls run concurrently instead of wasting half the array
        ps = _psB(pp, [128, 512])
        nc.tensor.matmul(
            ps[0:64, :], wc[:], hq[:, off : off + 512],
            start=True, stop=True, tile_position=(0, 0),
        )
        nc.tensor.matmul(
            ps[64:128, :], wc[:], hq[:, off + 512 : off + 1024],
            start=True, stop=True, tile_position=(0, 64),
        )
        nc.vector.tensor_scalar_add(moq[0:C, off : off + 512], ps[0:C, :], b2c[0:C, :])
        nc.vector.tensor_scalar_add(
            moq[0:C, off + 512 : off + 1024], ps[64 : 64 + C, :], b2c[0:C, :]
        )
        nc.scalar.activation(
            moq[C : 2 * C, off : off + 512],
            ps[C : 2 * C, :],
            AF.Exp,
            bias=b2c[C : 2 * C, :],
        )
        nc.scalar.activation(
            moq[C : 2 * C, off + 512 : off + 1024],
            ps[64 + C : 128, :],
            AF.Exp,
            bias=b2c[C : 2 * C, :],
        )
    nc.gpsimd.dma_start(
        mo[b][:, v0 : v0 + VQ, :],
        moq[:].rearrange("o (v l) -> o v l", v=VQ),
    )


def _ph2_loads(nc, pw, mo, b, g):
    """Load embed and expP node-major tiles for one c-group."""
    c0 = g * G
    egs, xps = [], []
    for i, (v0, sz) in enumerate(VT):
        eg = pw.tile([sz, G * L], F16, tag=f"eg{i}", name=f"eg{i}", bufs=2)
        xp = pw.tile([sz, G * L], F16, tag=f"xp{i}", name=f"xp{i}", bufs=2)
        nc.sync.dma_start(
            eg[:].rearrange("v (c l) -> v c l", c=G),
            mo[b][c0 : c0 + G, v0 : v0 + sz, :].rearrange("c v l -> v c l"),
        )
        nc.sync.dma_start(
            xp[:].rearrange("v (c l) -> v c l", c=G),
            mo[b][C + c0 : C + c0 + G, v0 : v0 + sz, :].rearrange("c v l -> v c l"),
        )
        egs.append(eg)
        xps.append(xp)
    return egs, xps


def _bcast(ap2d, nfree):
    """Append a stride-0 free dim of size nfree to a [p, c] AP."""
    return AP(ap2d.tensor, ap2d.offset, list(ap2d.ap) + [[0, nfree]])


def _ph2_group(nc, pw, pp, st, ld, dvall, xn_out, ant_out, b, g, ev):
    amt, idk = st["amt"], st["idk"]
    egs, xps = ld
    c0 = g * G

    SW = N + 2  # even channel stride for the s^T-raw + D layout (psum 8B lines)
    xraw = [
        pw.tile([sz, G * L], F16, tag=f"xr{i}", name=f"xr{i}", bufs=2)
        for i, (_, sz) in enumerate(VT)
    ]
    slgr = [
        pw.tile([sz, G * SW], F16, tag=f"sr{i}", name=f"sr{i}", bufs=2)
        for i, (_, sz) in enumerate(VT)
    ]
    slg = [
        pw.tile([sz, G * N], F16, tag=f"sl{i}", name=f"sl{i}", bufs=2)
        for i, (_, sz) in enumerate(VT)
    ]
    ttg = [
        pw.tile([sz, G * N], F16, tag=f"tt{i}", name=f"tt{i}", bufs=2)
        for i, (_, sz) in enumerate(VT)
    ]
    angr = [
        pw.tile([sz, G * N], F16, tag=f"ag{i}", name=f"ag{i}", bufs=2)
        for i, (_, sz) in enumerate(VT)
    ]

    # ---- s^T raw (+ D in col 160) via matmul vs identity|ones rhs ----
    for lt, (l0, lsz) in enumerate(VT):  # output l tiles
        for t0, nch in TRIPLES:
            ps = _psB(pp, [128, 512])
            for j in range(nch):
                ci = t0 + j
                for kt, (k0, ksz) in enumerate(VT):  # v tiles (contraction)
                    nc.tensor.matmul(
                        ps[:lsz, j * SW : (j + 1) * SW],
                        xps[kt][:, ci * L + l0 : ci * L + l0 + lsz],
                        idk[kt][:],
                        start=(kt == 0),
                        stop=(kt == 1),
                    )
            ev.copy(
                slgr[lt][:lsz, t0 * SW : (t0 + nch) * SW], ps[:lsz, 0 : nch * SW]
            )
        # one strided reciprocal per l-tile: Dinv for all 8 channels
        nc.vector.reciprocal(
            dvall[lt][:lsz, c0 : c0 + G],
            slgr[lt][:lsz].rearrange("p (c q) -> p c q", c=G)[:, :, N],
        )
        nc.vector.tensor_tensor(
            slg[lt][:lsz].rearrange("p (c v) -> p c v", c=G),
            slgr[lt][:lsz].rearrange("p (c q) -> p c q", c=G)[:, :, 0:N],
            _bcast(dvall[lt][:lsz, c0 : c0 + G], N),
            MUL,
        )

    # ---- tT = a^T s^T ----
    for mi, (m0, msz) in enumerate(VT):  # j tiles
        for cf0, csz in ((0, 512), (512, 512), (1024, 256)):
            ps = _psB(pp, [128, 512])
            for kt in range(2):
                nc.tensor.matmul(
                    ps[:msz, :csz],
                    amt[kt][mi][:],
                    slg[kt][:, cf0 : cf0 + csz],
                    start=(kt == 0),
                    stop=(kt == 1),
                )
            ev.copy(ttg[mi][:msz, cf0 : cf0 + csz], ps[:msz, :csz])

    # ---- x_new raw + a_new^T raw, interleaved: adjacent matmuls share the
    # same expP stationary slice, doubling moving data per weight load ----
    for mi, (m0, msz) in enumerate(VT):  # l tiles (output partition)
        for t0, nch in TRIPLES:
            psx = _psA(pp, [128, 512])
            psa = _psA(pp, [128, 512])
            for j in range(nch):
                ci = t0 + j
                for kt, (k0, ksz) in enumerate(VT):  # v/j tiles (contraction)
                    stat = xps[kt][:, ci * L + m0 : ci * L + m0 + msz]
                    nc.tensor.matmul(
                        psx[:msz, j * L : (j + 1) * L],
                        stat,
                        egs[kt][:, ci * L : (ci + 1) * L],
                        start=(kt == 0),
                        stop=(kt == 1),
                    )
                    nc.tensor.matmul(
                        psa[:msz, j * N : (j + 1) * N],
                        stat,
                        ttg[kt][:, ci * N : (ci + 1) * N],
                        start=(kt == 0),
                        stop=(kt == 1),
                    )
            ev.copy(xraw[mi][:msz, t0 * L : (t0 + nch) * L], psx[:msz, 0 : nch * L])
            ev.copy(angr[mi][:msz, t0 * N : (t0 + nch) * N], psa[:msz, 0 : nch * N])

    # ---- outputs (raw fp16; host applies Dinv scale / transpose) ----
    for i, (v0, sz) in enumerate(VT):
        nc.scalar.dma_start(
            xn_out[b][c0 : c0 + G, v0 : v0 + sz, :].rearrange("c p q -> p c q"),
            xraw[i][:sz].rearrange("p (c q) -> p c q", c=G),
        )
        nc.scalar.dma_start(
            ant_out[b][c0 : c0 + G, v0 : v0 + sz, :].rearrange("c p q -> p c q"),
            angr[i][:sz].rearrange("p (c q) -> p c q", c=G),
        )


def _host_prep(x, a, We, be, Wp, bp):
    a = np.asarray(a, np.float64)
    I = np.eye(N, dtype=np.float64)
    A1 = (a + I) / (a + I).sum(1, keepdims=True)
    A2 = (a.T + I) / (a.T + I).sum(1, keepdims=True)
    M1 = A1 + A2
    M2 = A1 @ A1 + A2 @ A2
    MT = np.concatenate([M1.T, M2.T], axis=1).astype(np.float16)  # [N, 2N]

    def fold(W):
        W = np.asarray(W, np.float64)
        W0, W1, W2 = W[:, :C], W[:, C : 2 * C], W[:, 2 * C :]
        F0 = 2.0 * (W0 + ALPHA * W1 + ALPHA * W2)
        F1 = BETA * W1 + ALPHA * BETA * W2
        F2 = BETA * BETA * W2
        return F0, F1, F2

    E0, E1, E2 = fold(We)
    P0, P1, P2 = fold(Wp)
    Wcat = np.block([[E0.T, P0.T], [E1.T, P1.T], [E2.T, P2.T]]).astype(np.float16)
    b2 = np.concatenate(
        [2.0 * np.asarray(be), 2.0 * np.asarray(bp) - LN_SHIFT]
    ).astype(np.float32)[:, None]
    return MT, Wcat, b2, np.asarray(a, np.float16)


def _postprocess(xn_raw, ant_raw, dv):
    # dv: [*, L, C] Dinv values; raw outputs are scaled by Dinv along their
    # l (row) dim, then a_new^T is transposed back.
    dinv = dv.transpose(0, 2, 1)[:, :, :, None]  # [*, C, L, 1]
    xn = xn_raw.astype(np.float32) * dinv
    an = (ant_raw.astype(np.float32) * dinv).swapaxes(-1, -2)
    return np.ascontiguousarray(xn), np.ascontiguousarray(an)


def _install_ntff_shim():
    """Provide antenv.axon_hooks (missing in this image) so
    run_bass_kernel_spmd(trace=True) can drive NTFF profiling via the
    axon PJRT .so. No-op if anything is unavailable."""
    import contextlib
    import ctypes
    import types

    try:
        import antenv  # noqa: F401

        try:
            from antenv.axon_hooks import get_axon_ntff_profile_hook  # noqa: F401

            return
        except ImportError:
            pass
        lib = ctypes.CDLL("/opt/axon/libaxon_pjrt.so")
        if not hasattr(lib, "axon_start_nrt_profile"):
            return
        lib.axon_start_nrt_profile.argtypes = [
            ctypes.POINTER(ctypes.c_int64),
            ctypes.c_size_t,
        ]
        lib.axon_start_nrt_profile.restype = ctypes.c_int64
        lib.axon_stop_nrt_profile.argtypes = [ctypes.c_char_p]
        lib.axon_stop_nrt_profile.restype = ctypes.c_int64

        @contextlib.contextmanager
        def _hook(output_dir, device_ids):
            import jax

            jax.devices()
            if device_ids:
                ids = (ctypes.c_int64 * len(device_ids))(*device_ids)
                rc = lib.axon_start_nrt_profile(ids, len(device_ids))
            else:
                rc = lib.axon_start_nrt_profile(None, 0)
            if rc != 0:
                raise RuntimeError(f"axon_start_nrt_profile rc={rc}")
            try:
                yield
            finally:
                n = lib.axon_stop_nrt_profile(str(output_dir).encode())
                print(f"ntff profile: {n} file(s) -> {output_dir}", file=sys.stderr)

        holder = {"h": _hook}
        mod = types.ModuleType("antenv.axon_hooks")
        mod.get_axon_ntff_profile_hook = lambda: holder["h"]
        mod.set_axon_ntff_profile_hook = lambda h: holder.__setitem__("h", h)
        sys.modules["antenv.axon_hooks"] = mod
        antenv.axon_hooks = mod
    except Exception as e:  # pragma: no cover
        print(f"ntff shim unavailable: {e}", file=sys.stderr)


_NC_CACHE = {}


def _get_nc():
    if "nc" not in _NC_CACHE:
        nc = build_nc()
        nc.compile()
        _NC_CACHE["nc"] = nc
    return _NC_CACHE["nc"]


def run_spmd(x, a, We, be, Wp, bp, trace=False):
    if trace:
        _install_ntff_shim()
    x16 = np.ascontiguousarray(np.asarray(x, np.float16))
    xnd = np.ascontiguousarray(x16.transpose(0, 2, 1, 3))  # [B, N, C, L]
    MT, Wcat, b2, a16 = _host_prep(x, a, We, be, Wp, bp)
    nc = _get_nc()
    in_maps = [
        {
            "xs": x16[i * BPC : (i + 1) * BPC],
            "xnd": xnd[i * BPC : (i + 1) * BPC],
            "mt": MT,
            "wcat": Wcat,
            "b2": b2,
            "am": a16,
        }
        for i in range(NCORES)
    ]
    res = run_bass_kernel_spmd(nc, in_maps, list(range(NCORES)), trace=trace)
    xn_raw = np.concatenate([res.results[i]["xn"] for i in range(NCORES)], axis=0)
    ant_raw = np.concatenate([res.results[i]["ant"] for i in range(NCORES)], axis=0)
    dv = np.concatenate([res.results[i]["dv"] for i in range(NCORES)], axis=0)
    xn, an = _postprocess(xn_raw, ant_raw, dv)
    return (xn, an), res


def kernel(x, a, We, be, Wp, bp):
    (xn, an), _ = run_spmd(x, a, We, be, Wp, bp, trace=False)
    return (xn, an)
